# revision 1
# baseline (speedup 1.0000x reference)
"""Trainium2 Bass kernel for nn_Memory_sup_33389075759209 (scatter_memory).

Strategy (8 NeuronCores, SPMD):
  - Data-parallel: core = b*2 + half. Each core processes one batch image's
    half (64 rows) extended to 68 rows (patch-aligned halo) so the whole
    pipeline through the 3x3 conv is core-local (no collectives).
  - The reference's huge M0 = m_items * mod intermediate ([B,5,128,H,W],
    320MB) is eliminated algebraically: M0c = conv1x1(mod, W2) with
    W2[o,(m,c)] = conv1_w[o,(m,c)] * m_items[m,c].
  - Phase-major execution so the ScalarEngine activation-table is switched
    only twice (sqrt set -> sigmoid set -> sqrt set).
  - bf16 matmuls (PE streams 1 col/cycle; fp32 would be 4x slower);
    float32r for the query projection (full-rate fp32 path, avoids a cast).
  - LayerNorms over the channel(partition) axis are done with ones/group
    matmuls for the reductions and K-small matmuls for the partition
    broadcast of the (rstd, -mean*rstd) pairs, with the LN affine weights
    folded into the broadcast matrices host-side.

kernel(**inputs) -> np.ndarray takes FULL inputs, shards, runs, gathers.
"""

import numpy as np
import ml_dtypes

B, C, H, W = 4, 128, 128, 128
M, P, DS = 5, 4, 4
R = 68            # extended rows per core
NBLK = R // 4     # 17 four-row blocks (= patch blocks)
NPATCH = NBLK * (W // P)   # 544 patches per core
NPIX = R * W      # 8704 pixels per core

_CACHE = {}


def _f32(x):
    return np.ascontiguousarray(np.asarray(x), dtype=np.float32)


def _bf16(x):
    return np.ascontiguousarray(np.asarray(x, dtype=np.float32).astype(ml_dtypes.bfloat16))


def _build_weights(m_items, mod_w, mod_b, conv1_w, conv1_b, conv2_w, conv2_b,
                   pe_w, pe_b, pe_g, pe_beta, exp_w, fin_g, fin_b, up_w, up_b,
                   wf_w2, wf_pre_w, wf_post_w, wf_bn_g, wf_bn_b):
    """Host-side algebraic folds. Returns dict name->np array for the kernel."""
    m_items = _f32(m_items); mod_w = _f32(mod_w); mod_b = _f32(mod_b)
    conv1_w = _f32(conv1_w); conv1_b = _f32(conv1_b)
    conv2_w = _f32(conv2_w); conv2_b = _f32(conv2_b)
    pe_w = _f32(pe_w); pe_b = _f32(pe_b); pe_g = _f32(pe_g); pe_beta = _f32(pe_beta)
    exp_w = _f32(exp_w); fin_g = _f32(fin_g); fin_b = _f32(fin_b)
    up_w = _f32(up_w); up_b = _f32(up_b)
    wf_w2 = _f32(wf_w2); wf_pre_w = _f32(wf_pre_w); wf_post_w = _f32(wf_post_w)
    wf_bn_g = _f32(wf_bn_g); wf_bn_b = _f32(wf_bn_b)

    ww = np.maximum(wf_w2, 0.0)
    fwt = ww / (ww.sum() + 1e-8)

    d = {}
    # ---- bf16 blob [128, 5256]: modT|w2T|c2T|peT|expT|g8|up4|w3T ----
    W2 = conv1_w.reshape(C // 2, M, C) * m_items[None, :, :]   # [64,5,128]
    peT = pe_w.transpose(2, 3, 1, 0).reshape(P * P, C, C)       # [(p,q), c, o]
    G8 = np.zeros((C, 8), np.float32)
    for k in range(C):
        G8[k, k // 32] = 1.0
        G8[k, 4 + k // 32] = 1.0
    ww_f = np.maximum(wf_w2, 0.0)
    fwt = ww_f / (ww_f.sum() + 1e-8)
    upf = up_w * fwt[1]
    up4 = np.zeros((C, 4 * C), np.float32)
    for d2 in range(4):
        up4[d2 * 32:(d2 + 1) * 32, d2 * C:(d2 + 1) * C] = upf.T
    gbn = wf_bn_g / np.sqrt(1.0 + 1e-5)
    w3 = wf_post_w * gbn[:, None, None, None]
    w3T = w3.transpose(2, 3, 1, 0).reshape(9, C, C)
    bf_parts = {
        'modT': mod_w.T,                                        # 640
        'w2T': W2.transpose(2, 1, 0).reshape(C, M * (C // 2)),  # 320
        'c2T': conv2_w.T,                                       # 64
        'peT': peT.transpose(1, 0, 2).reshape(C, P * P * C),    # 2048
        'expT': exp_w,                                          # 512
        'g8': G8,                                               # 8
        'up4': up4,                                             # 512
        'w3T': w3T.transpose(1, 0, 2).reshape(C, 9 * C),        # 1152
        'ln': None,                                             # 512 (placeholder)
    }
    lnblk = np.zeros((C, 512), np.float32)
    lnblk[0, 0:C] = pe_g
    lnblk[0, C:2 * C] = -pe_g
    ln2m = np.zeros((4, 2 * C), np.float32)
    for m in range(C):
        ln2m[m // 32, m] = fin_g[m % 32]
        ln2m[m // 32, C + m] = -fin_g[m % 32]
    lnblk[0:4, 256:512] = ln2m
    bf_parts['ln'] = lnblk
    bf_off = {}
    cols = []
    off = 0
    for k, v in bf_parts.items():
        bf_off[k] = off
        off += v.shape[1]
        cols.append(np.asarray(v, np.float32))
    d['w_bf'] = _bf16(np.concatenate(cols, axis=1))
    d['_bf_off'] = bf_off

    # ---- f32 blob [128, 528] layout:
    #  cols 0:5   modb (per m)
    #  col  5     snb
    #  col  6     peb
    #  col  7     pebeta
    #  col  8     upb
    #  col  9     bnb
    #  cols 16:272  ln1 rows0 (A|B) at partition 0 only
    #  cols 272:528 ln2 rows0:4 (A|B)
    f32 = np.zeros((C, 528), np.float32)
    f32[:, 0:5] = mod_b.reshape(M, C).T
    f32[:, 5] = np.concatenate([conv1_b, conv2_b])
    f32[:, 6] = pe_b
    f32[:, 7] = pe_beta
    f32[:, 8] = fwt[1] * up_b + upf @ fin_b
    f32[:, 9] = wf_bn_b
    f32[0, 16:16 + C] = pe_g
    f32[0, 16 + C:16 + 2 * C] = -pe_g
    ln2 = np.zeros((4, 2 * C), np.float32)
    for m in range(C):
        ln2[m // 32, m] = fin_g[m % 32]
        ln2[m // 32, C + m] = -fin_g[m % 32]
    f32[0:4, 272:272 + 2 * C] = ln2
    d['w_f32'] = _f32(f32)
    d['w_preT'] = _f32(wf_pre_w.T * fwt[0])
    return d


def _patch_act_tables():
    """Pin Ln/Exp to the natural_log_exp_and_others ACT table set so the
    compiler emits 3 table loads total instead of ping-ponging per tile."""
    import functools
    import concourse.hw_specs as hw_specs
    import concourse.bacc as bacc_mod
    import concourse.mybir as mybir
    if getattr(hw_specs.get_activation_tables, '_ln_exp_pinned', False):
        return
    _orig = hw_specs.get_activation_tables

    @functools.cache
    def patched(arch):
        t = {k: set(v) for k, v in _orig(arch).items()}
        AF = mybir.ActivationFunctionType
        for name, fns in t.items():
            if name != 'natural_log_exp_and_others':
                fns.discard(AF.Ln)
                fns.discard(AF.Exp)
        return t

    patched._ln_exp_pinned = True
    hw_specs.get_activation_tables = patched
    bacc_mod.get_activation_tables = patched


MW = 2          # macro width in 4-row blocks (512px each)
MW0 = 2         # phase-0 macro width


def _build_program(modb_zero):
    import concourse.bass as bass
    import concourse.bacc as bacc
    import concourse.tile as tile
    import concourse.mybir as mybir
    _patch_act_tables()

    dt = mybir.dt
    AF = mybir.ActivationFunctionType
    OP = mybir.AluOpType
    F32, BF16, F32R = dt.float32, dt.bfloat16, dt.float32r

    nc = bacc.Bacc('TRN2', target_bir_lowering=False, debug=False, num_devices=8)

    St_d = nc.dram_tensor('x_st', [C, R, W], F32, kind='ExternalInput').ap()
    Q_d = nc.dram_tensor('x_q', [C, R, W], F32R, kind='ExternalInput').ap()
    NBF = 5768
    Wbf_d = nc.dram_tensor('w_bf', [C, NBF], BF16, kind='ExternalInput').ap()
    Wf_d = nc.dram_tensor('w_f32', [C, 528], F32, kind='ExternalInput').ap()
    Wp_d = nc.dram_tensor('w_preT', [C, C], F32R, kind='ExternalInput').ap()
    Y_d = nc.dram_tensor('y_out', [C, R, W], F32, kind='ExternalOutput').ap()
    BO = {'modT': 0, 'w2T': 640, 'c2T': 960, 'peT': 1024, 'expT': 3072,
          'g8': 3584, 'up4': 3592, 'w3T': 4104, 'ln': 5256}

    with tile.TileContext(nc) as tc:
        with (
            tc.tile_pool(name='singles', bufs=1) as singles,
            tc.tile_pool(name='big', bufs=1) as big,
        ):
            # ---- load weights to SBUF (3 packed DMAs) ----
            bfw = singles.tile([C, NBF], BF16)
            nc.sync.dma_start(out=bfw[:], in_=Wbf_d[:])
            f32w = singles.tile([C, 528], F32)
            nc.sync.dma_start(out=f32w[:], in_=Wf_d[:])
            preT = singles.tile([C, C], F32R)
            nc.sync.dma_start(out=preT[:], in_=Wp_d[:])
            wsb = {
                'w_modT': bfw[:, BO['modT']:BO['modT'] + M * C],
                'w_w2T': bfw[:, BO['w2T']:BO['w2T'] + M * 64],
                'w_c2T': bfw[:, BO['c2T']:BO['c2T'] + 64],
                'w_peT': bfw[:, BO['peT']:BO['peT'] + 16 * C],
                'w_expT': bfw[:, BO['expT']:BO['expT'] + DS * C],
                'w_g8': bfw[:, BO['g8']:BO['g8'] + 8],
                'w_up4': bfw[:, BO['up4']:BO['up4'] + 4 * C],
                'w_w3T': bfw[:, BO['w3T']:BO['w3T'] + 9 * C],
                'w_modb': f32w[:, 0:5],
                'w_snb': f32w[:, 5:6],
                'w_peb': f32w[:, 6:7],
                'w_pebeta': f32w[:, 7:8],
                'w_upb': f32w[:, 8:9],
                'w_bnb': f32w[:, 9:10],
                'w_ln1': bfw[0:1, BO['ln']:BO['ln'] + 256],
                'w_ln2': bfw[0:4, BO['ln'] + 256:BO['ln'] + 512],
                'w_preT': preT[:],
            }
            ones_bf = singles.tile([C, C], BF16)
            nc.vector.memset(ones_bf, 1.0)
            ones1_bf = ones_bf[:, 0:1]
            tiny_t = singles.tile([C, 1], dt.float32)
            nc.vector.memset(tiny_t, 1e-30)
            eps_t = singles.tile([C, 1], dt.float32)
            nc.vector.memset(eps_t, 1e-5)

            # ---- persistent activations ----
            s_sb = big.tile([C, NPIX], BF16)       # normalized structure
            sn_sb = big.tile([C, NPIX], BF16)      # Sn (concat M0c, Sc)
            f_sb = big.tile([C, NPATCH], BF16)     # patch embed out (pre-LN)
            fln_sb = big.tile([C, NPATCH], BF16)   # post-LN1
            fe_sb = big.tile([C, DS * NPATCH], BF16)    # expand out (pre-LN2), [c, d1, patch]
            feln_sb = big.tile([C, DS * NPATCH], BF16)  # post-LN2
            x_pad = big.tile([C, 70 * 130], BF16)  # conv input, zero padded
            xv = x_pad.rearrange("c (r w) -> c r w", r=70)
            nc.vector.memset(xv[:, 0, :], 0.0)
            nc.vector.memset(xv[:, 69, :], 0.0)
            nc.vector.memset(xv[:, 1:69, 0], 0.0)
            nc.vector.memset(xv[:, 1:69, 129], 0.0)

            MACROS = [(i, min(i + MW, NBLK)) for i in range(0, NBLK, MW)]
            MAXW = 512 * MW
            PSB = max(1, min(4, 8 // (2 * MW)))   # psum double-buffer count
            MACROS0 = [(i, min(i + MW0, NBLK)) for i in range(0, NBLK, MW0)]
            # ================= Phase 0: l2norm (ln/exp table) ============
            with (
                tc.tile_pool(name='p0', bufs=3) as p0,
                tc.tile_pool(name='p0ps', bufs=min(4, 8 // MW), space='PSUM') as p0ps,
            ):
                for (b0, b1) in MACROS0:
                    n = 512 * (b1 - b0)
                    st_t = p0.tile([C, MAXW], F32, tag='st')
                    nc.sync.dma_start(out=st_t[:, 0:n], in_=St_d[:, 4 * b0:4 * b1, :])
                    stb_t = p0.tile([C, MAXW], BF16, tag='stb')
                    nc.vector.tensor_copy(stb_t[:, 0:n], st_t[:, 0:n])
                    sq_t = p0.tile([C, MAXW], BF16, tag='sq')
                    nc.vector.tensor_mul(sq_t[:, 0:n], stb_t[:, 0:n], stb_t[:, 0:n])
                    ps = p0ps.tile([C, MAXW], F32, tag='ss')
                    for j in range(b1 - b0):
                        nc.tensor.matmul(ps[:, 512 * j:512 * (j + 1)], ones_bf[:],
                                         sq_t[:, 512 * j:512 * (j + 1)],
                                         start=True, stop=True)
                    lg_t = p0.tile([C, MAXW], F32, tag='lg')
                    nc.scalar.activation(lg_t[:, 0:n], ps[:, 0:n], AF.Ln, bias=tiny_t[:])
                    rst_t = p0.tile([C, MAXW], BF16, tag='rst')
                    nc.scalar.activation(rst_t[:, 0:n], lg_t[:, 0:n], AF.Exp, scale=-0.5)
                    nc.vector.tensor_mul(s_sb[:, 512 * b0:512 * b0 + n],
                                         stb_t[:, 0:n], rst_t[:, 0:n])

            # 2f: query projection -> x_pad
            with (
                tc.tile_pool(name='p2f', bufs=2) as p2f,
                tc.tile_pool(name='p2fps', bufs=min(4, 8 // MW), space='PSUM') as p2fps,
            ):
                for (b0, b1) in MACROS:
                    nb = b1 - b0
                    n = 512 * nb
                    q_t = p2f.tile([C, MAXW], F32R, tag='q')
                    nc.sync.dma_start(out=q_t[:, 0:n], in_=Q_d[:, 4 * b0:4 * b1, :])
                    psq = p2fps.tile([C, MAXW], F32, tag='psq')
                    for j in range(nb):
                        nc.tensor.matmul(psq[:, 512 * j:512 * (j + 1)], wsb['w_preT'][:],
                                         q_t[:, 512 * j:512 * (j + 1)], start=True, stop=True)
                    nc.scalar.activation(
                        xv[:, 1 + 4 * b0:1 + 4 * b1, 1:129],
                        psq[:, 0:n].rearrange("c (r w) -> c r w", w=W), AF.Copy)




            # ================= Phase 1: modulate (sigmoid table) =========
            with (
                tc.tile_pool(name='p1', bufs=2) as p1,
                tc.tile_pool(name='p1psA', bufs=PSB, space='PSUM') as p1psA,
                tc.tile_pool(name='p1psC', bufs=PSB, space='PSUM') as p1psC,
            ):
                for (b0, b1) in MACROS:
                    nb = b1 - b0
                    n = 512 * nb
                    s_blk = s_sb[:, 512 * b0:512 * b0 + n]
                    sig_t = p1.tile([C, M * MAXW], BF16, tag='sig')
                    psm = p1psA.tile([C, MAXW], F32, tag='mod')
                    for m in range(M):
                        for j in range(nb):
                            nc.tensor.matmul(psm[:, 512 * j:512 * (j + 1)],
                                             wsb['w_modT'][:, C * m:C * (m + 1)],
                                             s_blk[:, 512 * j:512 * (j + 1)],
                                             start=True, stop=True)
                        nc.scalar.activation(sig_t[:, MAXW * m:MAXW * m + n], psm[:, 0:n],
                                             AF.Sigmoid, bias=wsb['w_modb'][:, m:m + 1])
                    # Sn: M0c into partitions 0:64, Sc into 64:128
                    psn = p1psC.tile([C, MAXW], F32, tag='sn')
                    for m in range(M):
                        for j in range(nb):
                            nc.tensor.matmul(psn[0:64, 512 * j:512 * (j + 1)],
                                             wsb['w_w2T'][:, 64 * m:64 * (m + 1)],
                                             sig_t[:, MAXW * m + 512 * j:MAXW * m + 512 * (j + 1)],
                                             start=(m == 0), stop=(m == M - 1))
                    for j in range(nb):
                        nc.tensor.matmul(psn[64:128, 512 * j:512 * (j + 1)], wsb['w_c2T'][:],
                                         s_blk[:, 512 * j:512 * (j + 1)],
                                         start=True, stop=True, tile_position=(0, 64))
                    nc.vector.tensor_scalar_add(sn_sb[:, 512 * b0:512 * b0 + n],
                                                psn[:, 0:n], wsb['w_snb'][:])

            # ================= Phase 2: patch/LN/expand/LN/up/qpre =======
            snv = sn_sb.rearrange("c (pb p ww q) -> c pb p ww q", pb=NBLK, p=P, q=P)

            # 2a: patch embed (groups of 16 patch-blocks -> N=512 matmuls)
            with tc.tile_pool(name='p2aps', bufs=2, space='PSUM') as p2aps:
                for (b0, b1) in [(0, 9), (9, 17)]:
                    npb = b1 - b0
                    n = 32 * npb
                    psf = p2aps.tile([C, 512], F32, tag='f')
                    for p in range(P):
                        for q in range(P):
                            nc.tensor.matmul(psf[:, 0:n],
                                             wsb['w_peT'][:, (p * 4 + q) * C:(p * 4 + q + 1) * C],
                                             snv[:, b0:b1, p, :, q],
                                             start=(p == 0 and q == 0),
                                             stop=(p == P - 1 and q == P - 1))
                    nc.scalar.activation(f_sb[:, 32 * b0:32 * b0 + n], psf[:, 0:n],
                                         AF.Identity, bias=wsb['w_peb'][:])

            # 2b: LN1 (over channel partitions; 2 chunks of 272)
            with (
                tc.tile_pool(name='p2b', bufs=2) as p2b,
                tc.tile_pool(name='p2bps', bufs=2, space='PSUM') as p2bps,
                tc.tile_pool(name='p2bps2', bufs=2, space='PSUM') as p2bps2,
            ):
                for ch in range(2):
                    n0, n1 = 272 * ch, 272 * (ch + 1)
                    fch = f_sb[:, n0:n1]
                    sqf = p2b.tile([C, 272], BF16, tag='sqf')
                    nc.vector.tensor_mul(sqf[:], fch, fch)
                    pstf = p2bps.tile([1, 272], F32, tag='st1f')
                    pstq = p2bps.tile([1, 272], F32, tag='st1q')
                    nc.tensor.matmul(pstf[:], ones1_bf, fch, start=True, stop=True)
                    nc.tensor.matmul(pstq[:], ones1_bf, sqf[:], start=True, stop=True)
                    mu = p2b.tile([1, 272], F32, tag='mu1')
                    nc.vector.tensor_scalar_mul(mu[:], pstf[:], 1.0 / C)
                    musq = p2b.tile([1, 272], F32, tag='musq1')
                    nc.vector.tensor_mul(musq[:], mu[:], mu[:])
                    var = p2b.tile([1, 272], F32, tag='var1')
                    nc.vector.scalar_tensor_tensor(var[:], pstq[:], 1.0 / C, musq[:],
                                                   op0=OP.mult, op1=OP.subtract)
                    sd = p2b.tile([1, 272], F32, tag='sd1')
                    nc.scalar.activation(sd[:], var[:], AF.Ln, bias=eps_t[0:1, :])
                    r_t = p2b.tile([1, 272], BF16, tag='r1')
                    nc.scalar.activation(r_t[:], sd[:], AF.Exp, scale=-0.5)
                    mur_t = p2b.tile([1, 272], BF16, tag='mur1')
                    nc.vector.tensor_mul(mur_t[:], mu[:], r_t[:])
                    psA = p2bps2.tile([C, 272], F32, tag='A1')
                    psB = p2bps2.tile([C, 272], F32, tag='B1')
                    nc.tensor.matmul(psA[:], wsb['w_ln1'][:, 0:C], r_t[:], start=True, stop=True)
                    nc.tensor.matmul(psB[:], wsb['w_ln1'][:, C:2 * C], mur_t[:], start=True, stop=True)
                    t1 = p2b.tile([C, 272], BF16, tag='t1')
                    nc.vector.tensor_mul(t1[:], fch, psA[:])
                    nc.vector.scalar_tensor_tensor(fln_sb[:, n0:n1], t1[:], wsb['w_pebeta'][:],
                                                   psB[:], op0=OP.add, op1=OP.add)

            # 2c: expand (fe[c, d1, patch])
            fev = fe_sb.rearrange("c (d n) -> c d n", d=DS)
            with tc.tile_pool(name='p2cps', bufs=3, space='PSUM') as p2cps:
                for d1 in range(DS):
                    for ch in range(2):
                        n0, n1 = 272 * ch, 272 * (ch + 1)
                        pse = p2cps.tile([C, 272], F32, tag='fe')
                        nc.tensor.matmul(pse[:], wsb['w_expT'][:, C * d1:C * (d1 + 1)],
                                         fln_sb[:, n0:n1], start=True, stop=True)
                        nc.scalar.activation(fev[:, d1, n0:n1], pse[:], AF.Copy)

            # 2d: LN2 over c' groups of 32 (chunks of 512 over 2176)
            ln2_chunks = [(i * 512, min((i + 1) * 512, DS * NPATCH)) for i in range(5)]
            with (
                tc.tile_pool(name='p2d', bufs=2) as p2d,
                tc.tile_pool(name='p2dps', bufs=1, space='PSUM') as p2dps,
                tc.tile_pool(name='p2dps2', bufs=2, space='PSUM') as p2dps2,
            ):
                for (n0, n1) in ln2_chunks:
                    n = n1 - n0
                    fch = fe_sb[:, n0:n1]
                    sqf = p2d.tile([C, 512], BF16, tag='sqf2')
                    nc.vector.tensor_mul(sqf[:, 0:n], fch, fch)
                    sta = p2dps.tile([4, 512], F32, tag='sta')
                    stb = p2dps.tile([4, 512], F32, tag='stb')
                    nc.tensor.matmul(sta[:, 0:n], wsb['w_g8'][:, 0:4], fch, start=True, stop=True)
                    nc.tensor.matmul(stb[:, 0:n], wsb['w_g8'][:, 4:8], sqf[:, 0:n], start=True, stop=True)
                    mu = p2d.tile([4, 512], F32, tag='mu2')
                    nc.vector.tensor_scalar_mul(mu[:, 0:n], sta[:, 0:n], 1.0 / 32)
                    musq = p2d.tile([4, 512], F32, tag='musq2')
                    nc.vector.tensor_mul(musq[:, 0:n], mu[:, 0:n], mu[:, 0:n])
                    var = p2d.tile([4, 512], F32, tag='var2')
                    nc.vector.scalar_tensor_tensor(var[:, 0:n], stb[:, 0:n], 1.0 / 32,
                                                   musq[:, 0:n], op0=OP.mult, op1=OP.subtract)
                    sd = p2d.tile([4, 512], F32, tag='sd2')
                    nc.scalar.activation(sd[:, 0:n], var[:, 0:n], AF.Ln, bias=eps_t[0:4, :])
                    ab_r = p2d.tile([4, 512], BF16, tag='ab2r')
                    ab_m = p2d.tile([4, 512], BF16, tag='ab2m')
                    nc.scalar.activation(ab_r[:, 0:n], sd[:, 0:n], AF.Exp, scale=-0.5)
                    nc.vector.tensor_mul(ab_m[:, 0:n], mu[:, 0:n], ab_r[:, 0:n])
                    psA = p2dps2.tile([C, 512], F32, tag='A2')
                    psB = p2dps2.tile([C, 512], F32, tag='B2')
                    nc.tensor.matmul(psA[:, 0:n], wsb['w_ln2'][:, 0:C], ab_r[:, 0:n],
                                     start=True, stop=True)
                    nc.tensor.matmul(psB[:, 0:n], wsb['w_ln2'][:, C:2 * C], ab_m[:, 0:n],
                                     start=True, stop=True)
                    t1 = p2d.tile([C, 512], BF16, tag='t2')
                    nc.vector.tensor_mul(t1[:, 0:n], fch, psA[:, 0:n])
                    nc.vector.tensor_add(feln_sb[:, n0:n1], t1[:, 0:n], psB[:, 0:n])

            # ===== 2e: up projection scatter-add, interleaved with conv ==
            xs = x_pad.rearrange("c (r w) -> c r w", r=70)[:, 1:69, 1:129] \
                      .rearrange("c (pb p) (ww q) -> c pb p ww q", p=P, q=P)
            felv = feln_sb.rearrange("c (d n) -> c d n", d=DS)
            with (
                tc.tile_pool(name='p2eps', bufs=2, space='PSUM') as p2eps,
                tc.tile_pool(name='p3', bufs=3) as p3,
                tc.tile_pool(name='p3ps', bufs=4, space='PSUM') as p3ps,
            ):
                def up_range(pb0, pb1):
                    npb = pb1 - pb0
                    n = npb * 32
                    for d2 in range(4):
                        for d1 in range(4):
                            psm = p2eps.tile([C, 512], F32, tag='m1', name='psm')
                            nc.tensor.matmul(psm[:, 0:n],
                                             wsb['w_up4'][:, C * d2:C * (d2 + 1)],
                                             felv[:, d1, 32 * pb0:32 * pb0 + n],
                                             start=True, stop=True)
                            dst = xs[:, pb0:pb1, d1, :, d2]
                            srcv = psm[:, 0:n].rearrange("c (pb ww) -> c pb ww", pb=npb)
                            nc.vector.scalar_tensor_tensor(dst, srcv, wsb['w_upb'][:], dst,
                                                           op0=OP.add, op1=OP.add)

                def conv_group(t0, t1):
                    psys = []
                    for t in range(t0, t1):
                        psy = p3ps.tile([C, 512], F32, tag='y', name=f'psy{t}')
                        psys.append(psy)
                    k = 0
                    for dr in range(3):
                        for dw in range(3):
                            wsl = wsb['w_w3T'][:, C * (dr * 3 + dw):C * (dr * 3 + dw + 1)]
                            for i in range(t1 - t0):
                                nc.tensor.matmul(psys[i][:], wsl,
                                                 xv[:, 4 * (t0 + i) + dr:4 * (t0 + i) + dr + 4,
                                                    dw:dw + 128],
                                                 start=(k == 0), stop=(k == 8))
                            k += 1
                    nmac = 512 * (t1 - t0)
                    ys = p3.tile([C, 2048], F32, tag='ys', name='ys')
                    for i, t in enumerate(range(t0, t1)):
                        yb = p3.tile([C, 512], F32, tag='yb', name='yb')
                        nc.scalar.activation(yb[:], psys[i][:], AF.Relu, bias=wsb['w_bnb'][:])
                        nc.vector.tensor_scalar_min(ys[:, 512 * i:512 * (i + 1)], yb[:], 6.0)
                    nc.sync.dma_start(out=Y_d[:, 4 * t0:4 * t1, :],
                                      in_=ys[:, 0:nmac].rearrange("c (r w) -> c r w", w=W))

                up_range(0, 8)
                conv_group(0, 4)
                conv_group(4, 7)
                up_range(8, 16)
                conv_group(7, 11)
                conv_group(11, 15)
                up_range(16, 17)
                conv_group(15, 17)
    nc.compile()
    return nc


def _get_program(modb_zero):
    key = ('prog', modb_zero)
    if key not in _CACHE:
        _CACHE[key] = _build_program(modb_zero)
    return _CACHE[key]


def kernel(Structure, query, m_items, mod_w, mod_b, conv1_w, conv1_b,
           conv2_w, conv2_b, pe_w, pe_b, pe_g, pe_beta, exp_w, fin_g,
           fin_b, up_w, up_b, wf_w2, wf_pre_w, wf_post_w, wf_bn_g, wf_bn_b):
    import os
    from concourse import bass_utils

    wdict = _build_weights(m_items, mod_w, mod_b, conv1_w, conv1_b, conv2_w,
                           conv2_b, pe_w, pe_b, pe_g, pe_beta, exp_w, fin_g,
                           fin_b, up_w, up_b, wf_w2, wf_pre_w, wf_post_w,
                           wf_bn_g, wf_bn_b)
    wdict.pop('_bf_off', None)
    nc = _get_program(True)

    Structure = _f32(Structure)
    query = _f32(query)
    in_maps = []
    for core in range(8):
        b, half = core // 2, core % 2
        rs = 0 if half == 0 else H - R
        im = {'x_st': np.ascontiguousarray(Structure[b, :, rs:rs + R, :]),
              'x_q': np.ascontiguousarray(query[b, :, rs:rs + R, :])}
        im.update(wdict)
        in_maps.append(im)

    trace = bool(int(os.environ.get('BASS_KERNEL_TRACE', '0')))
    res = bass_utils.run_bass_kernel_spmd(nc, in_maps, core_ids=list(range(8)),
                                          trace=trace)
    _CACHE['last_results'] = res

    out = np.empty((B, C, H, W), np.float32)
    for core in range(8):
        b, half = core // 2, core % 2
        y = res.results[core]['y_out']
        if half == 0:
            out[b, :, 0:64, :] = y[:, 0:64, :]
        else:
            out[b, :, 64:128, :] = y[:, 4:68, :]
    return out

